# revision 1
# baseline (speedup 1.0000x reference)
"""Trainium2 Bass kernel for graph-contrastive BCE-with-logits loss.

Computes mean over nodes of per-node mean BCE loss:
  scores[n,k] = <x_n, x_{idx[n,k]}>   (16 pos + 16 neg neighbors per node)
  loss = mean_n [ (sum_k softplus(-s_pos) + sum_k softplus(s_neg)) / 32 ]

Strategy (8 NeuronCores, data-parallel over nodes):
  - Replicate node_features (plus one appended zero row) to every core as the
    gather table; each core processes a 12500-node shard padded to 12544
    (98 tiles of 128 nodes). Fake pad nodes use zero x rows and point at the
    zero table row -> contribute exactly 32*ln(2) each, subtracted on host.
  - Per tile: one indirect DMA gathers the 32 neighbor rows of 128 nodes
    (4096 descriptors x 512B) from HBM into SBUF, casting f32->bf16 in
    flight. VectorE does the elementwise multiply (bf16 2x mode) plus a
    3-level pairwise tree fold, then a reduce to per-(node,k) scores s.
  - softplus(sigma*s) is summed via identity softplus(-s) = softplus(s) - s:
    loss terms = relu(s) + ln(1+exp(-|s|)) summed over all 32 k, minus the
    sum of the 16 positive-column scores. ScalarE (ACT) computes
    abs/exp/ln/relu (all in the natural_log_exp table set) with accum_out
    writing per-tile partial sums into stage buffers.
  - Final per-core [128,1] partial sums are DMA'd out; host reduces the 8
    cores (the "all-reduce" of the scalar loss).
"""

import numpy as np

P = 128          # nodes per tile (partition dim)
D = 128          # feature dim
KP = 16          # positive neighbors
KN = 16          # negative neighbors
K = KP + KN      # 32 gathered rows per node

N_FULL = 100000
N_CORES = 8
SHARD = N_FULL // N_CORES            # 12500
N_TILES = -(-SHARD // P)             # 98
SHARD_PAD = N_TILES * P              # 12544
N_FAKE_PER_CORE = SHARD_PAD - SHARD  # 44


def build_program(n_tiles, table_rows, cast_in_gather=True):
    """Build the Bass/Tile program for one core's shard.

    Inputs (per core): table [table_rows, D] f32, x [n_tiles*P, D] f32,
    idx [P, n_tiles*K] int32 (idx[p, j*K+k] = global row id of the k-th
    neighbor of local node j*P+p; first KP columns of each K-group are
    positive neighbors).
    Output: out [P, 1] f32 -- per-partition sum over tiles of
    (sum_k softplus-terms - sum_pos s).
    """
    import concourse.bacc as bacc
    import concourse.bass as bass
    import concourse.mybir as mybir
    import concourse.tile as tile

    f32 = mybir.dt.float32
    bf16 = mybir.dt.bfloat16
    AF = mybir.ActivationFunctionType
    ALU = mybir.AluOpType
    AX = mybir.AxisListType

    nc = bacc.Bacc("TRN2", target_bir_lowering=False, debug=False)
    n_nodes = n_tiles * P
    table = nc.dram_tensor("table", [table_rows, D], f32, kind="ExternalInput")
    xin = nc.dram_tensor("x", [n_nodes, D], f32, kind="ExternalInput")
    idx = nc.dram_tensor("idx", [P, n_tiles * K], mybir.dt.int32, kind="ExternalInput")
    out = nc.dram_tensor("out", [P, 1], f32, kind="ExternalOutput")

    with tile.TileContext(nc) as tc:
        with (
            tc.tile_pool(name="const", bufs=1) as cpool,
            tc.tile_pool(name="gp", bufs=3) as gpool,
            tc.tile_pool(name="fp", bufs=2) as fpool,
            tc.tile_pool(name="sp", bufs=3) as spool,
        ):
            idx_sb = cpool.tile([P, n_tiles * K], mybir.dt.int32, tag="idx")
            nc.sync.dma_start(out=idx_sb[:], in_=idx[:])

            # x rows for node j*P+p land at xall[p, j*D:(j+1)*D], cast to bf16
            xall = cpool.tile([P, n_tiles * D], bf16, tag="xall")
            nc.gpsimd.dma_start(
                out=xall[:].rearrange("p (j d) -> p j d", d=D),
                in_=xin[:].rearrange("(j p) d -> p j d", p=P),
            )

            stage_u = cpool.tile([P, n_tiles], f32, tag="stu")
            stage_r = cpool.tile([P, n_tiles], f32, tag="str")
            stage_sp = cpool.tile([P, n_tiles], f32, tag="stsp")

            for j in range(n_tiles):
                gdt = bf16 if cast_in_gather else f32
                g = gpool.tile([P, K * D], gdt, tag="g")
                nc.gpsimd.indirect_dma_start(
                    out=g[:],
                    out_offset=None,
                    in_=table[:],
                    in_offset=bass.IndirectOffsetOnAxis(
                        ap=idx_sb[:, j * K : (j + 1) * K], axis=0
                    ),
                )
                if cast_in_gather:
                    gv = g[:].rearrange("p (k d) -> p k d", d=D)
                else:
                    gb = gpool.tile([P, K * D], bf16, tag="gb")
                    nc.scalar.activation(gb[:], g[:], AF.Copy)
                    gv = gb[:].rearrange("p (k d) -> p k d", d=D)

                xb = xall[:, j * D : (j + 1) * D]
                xbb = xb[:, None, :].to_broadcast([P, K, D])
                # products, in place over the gathered tile
                nc.vector.tensor_tensor(out=gv, in0=gv, in1=xbb, op=ALU.mult)
                # pairwise tree fold over d: 128 -> 64 -> 32 -> 16
                p2 = fpool.tile([P, K * 64], bf16, tag="p2")
                v2 = p2[:].rearrange("p (k d) -> p k d", d=64)
                nc.vector.tensor_tensor(
                    out=v2, in0=gv[:, :, 0:64], in1=gv[:, :, 64:128], op=ALU.add
                )
                p3 = fpool.tile([P, K * 32], bf16, tag="p3")
                v3 = p3[:].rearrange("p (k d) -> p k d", d=32)
                nc.vector.tensor_tensor(
                    out=v3, in0=v2[:, :, 0:32], in1=v2[:, :, 32:64], op=ALU.add
                )
                p4 = fpool.tile([P, K * 16], bf16, tag="p4")
                v4 = p4[:].rearrange("p (k d) -> p k d", d=16)
                nc.vector.tensor_tensor(
                    out=v4, in0=v3[:, :, 0:16], in1=v3[:, :, 16:32], op=ALU.add
                )
                s = spool.tile([P, K], f32, tag="s")
                nc.vector.tensor_reduce(out=s[:], in_=v4, axis=AX.X, op=ALU.add)
                # sum of positive-column scores (softplus(-s) = softplus(s) - s)
                nc.vector.tensor_reduce(
                    out=stage_sp[:, j : j + 1], in_=s[:, 0:KP], axis=AX.X, op=ALU.add
                )
                # stable softplus terms: relu(s) + ln(1 + exp(-|s|))
                a = spool.tile([P, K], f32, tag="a")
                nc.scalar.activation(a[:], s[:], AF.Abs)
                e = spool.tile([P, K], f32, tag="e")
                nc.scalar.activation(e[:], a[:], AF.Exp, scale=-1.0)
                u = spool.tile([P, K], f32, tag="u")
                nc.scalar.activation(
                    u[:], e[:], AF.Ln, bias=1.0, accum_out=stage_u[:, j : j + 1]
                )
                r = spool.tile([P, K], f32, tag="r")
                nc.scalar.activation(
                    r[:], s[:], AF.Relu, accum_out=stage_r[:, j : j + 1]
                )

            su = cpool.tile([P, 1], f32, tag="su")
            sr = cpool.tile([P, 1], f32, tag="sr")
            ssp = cpool.tile([P, 1], f32, tag="ssp")
            nc.vector.tensor_reduce(out=su[:], in_=stage_u[:], axis=AX.X, op=ALU.add)
            nc.vector.tensor_reduce(out=sr[:], in_=stage_r[:], axis=AX.X, op=ALU.add)
            nc.vector.tensor_reduce(out=ssp[:], in_=stage_sp[:], axis=AX.X, op=ALU.add)
            tot = cpool.tile([P, 1], f32, tag="tot")
            nc.vector.tensor_tensor(out=tot[:], in0=su[:], in1=sr[:], op=ALU.add)
            nc.vector.tensor_tensor(out=tot[:], in0=tot[:], in1=ssp[:], op=ALU.subtract)
            nc.sync.dma_start(out=out[:], in_=tot[:])

    nc.compile()
    return nc


def _prep_core_inputs(node_features, gi32, core):
    """Build one core's x slice (padded) and index layout."""
    lo, hi = core * SHARD, (core + 1) * SHARD
    x = np.zeros((SHARD_PAD, D), dtype=np.float32)
    x[:SHARD] = node_features[lo:hi]
    gi = np.full((SHARD_PAD, K), N_FULL, dtype=np.int32)
    gi[:SHARD] = gi32[lo:hi]
    idx_l = np.ascontiguousarray(
        gi.reshape(N_TILES, P, K).transpose(1, 0, 2).reshape(P, N_TILES * K)
    )
    return x, idx_l


_CACHED = {}


def kernel(node_features, pos_idx, neg_idx):
    node_features = np.asarray(node_features, dtype=np.float32)
    gi32 = np.concatenate(
        [np.asarray(pos_idx), np.asarray(neg_idx)], axis=1
    ).astype(np.int32)

    table = np.zeros((N_FULL + 1, D), dtype=np.float32)
    table[:N_FULL] = node_features

    in_maps = []
    for c in range(N_CORES):
        x, idx_l = _prep_core_inputs(node_features, gi32, c)
        in_maps.append({"table": table, "x": x, "idx": idx_l})

    if "nc" not in _CACHED:
        _CACHED["nc"] = build_program(N_TILES, N_FULL + 1)
    nc = _CACHED["nc"]

    from concourse import bass_utils

    res = bass_utils.run_bass_kernel_spmd(
        nc, in_maps, core_ids=list(range(N_CORES))
    )
    total = sum(float(np.asarray(r["out"], dtype=np.float64).sum()) for r in res.results)
    total -= N_CORES * N_FAKE_PER_CORE * K * float(np.log(2.0))
    mean = total / (N_FULL * K)
    return np.float32(mean)
